# revision 2
# baseline (speedup 1.0000x reference)
"""DeepSeek-style MoE block on 8 Trainium2 NeuronCores.

Sharding strategy (ff-dimension tensor-parallel over the routed experts):
  - Every core holds a 256-wide ff-shard (FF_DIM/8) of ALL 8 routed experts'
    weights and processes EVERY routed (token, expert) pair at 1/8 of the
    FFN work.  Per-core routed work is exactly sum(loads)/8 = 1024 token
    units regardless of the gate's load balance (the old expert-parallel
    layout paid max(load) ~= 1104 per core in padding).
  - The shared expert stays data-parallel: core c processes tokens
    [c*512,(c+1)*512) with the full shared weights.
  - Routed token batches are gathered per expert on host into slots sized
    to the exact per-expert load (rounded to 16), sorted descending so the
    final slot (smallest) minimizes the tail drain.
  - Device emits raw bf16 FFN partials (no gate scaling); the host sums the
    8 ff-shard partials, applies the normalized gate weights and the final
    /(N_SHARED+TOP_K) scale, and scatter-adds.  bf16 partials add ~1e-3
    relative error - well inside the 2e-2 gate (bf16 matmul floor is 4e-3).
  - Matmuls run in bf16 with fp32 PSUM accumulation.
"""

import os

import numpy as np
import ml_dtypes

import concourse.bass as bass
import concourse.mybir as mybir
import concourse.tile as tile_mod
from concourse.bass_utils import run_bass_kernel_spmd
from concourse.vector_clock import ScopedClock

_BF16 = ml_dtypes.bfloat16
P = 128
D_MODEL = 1024
FF_DIM = 2048
N_ROUTED = 8
TOP_K = 2
N_CORES = 8
T_TOKENS = 4096           # 2 * 2048
TS = T_TOKENS // N_CORES  # shared-expert tokens per core
KO1 = D_MODEL // P        # 8  k-chunks for d_model contraction / d_model out
FO_S = FF_DIM // P        # 16 ff chunks for the (full-ff) shared expert
FF_SH = FF_DIM // N_CORES  # 256-wide ff shard per core for routed experts
FO_R = FF_SH // P         # 2  ff chunks per routed slot
NT = 512                  # token tile (matmul moving free dim; one PSUM bank)

LAST_EXEC_NS = None       # set after each kernel() call when profiling


def _split_drain_and_barrier(self, tick_clock, wait_clock):
    """Tile's kernel-tail drain carries one sem-wait per touched engine/queue;
    this walrus build rejects >2 sync waits per instruction. Split the waits
    into single-wait sync nops ahead of the drain (semantically identical:
    the SP stream blocks on each wait in sequence before drain+barrier)."""
    nc = self.nc
    probe = nc.sync.nop(nofuse=True)
    wait_clock.add_sem_waits(probe.ins, ScopedClock({None: tick_clock.global_clock}))
    waits = list(probe.ins.sync_info.on_wait)
    if len(waits) > 1:
        probe.ins.sync_info.on_wait[:] = waits[:1]
        SyncInfo = type(probe.ins.sync_info)
        for w in waits[1:]:
            n2 = nc.sync.nop(nofuse=True)
            n2.ins.sync_info = SyncInfo(on_update=[], on_wait=[w])
    nc.sync.drain()
    nc.all_engine_barrier()
    popped = nc._tile_sem_poison_stack.pop()
    assert popped is self._sem_poison
    nc.clear_and_free_semaphores(list(self.sems.allocated().values()))
    nc.all_engine_barrier()


_MAX_WAITS = 1  # this walrus build rejects multiple sync waits on one instruction


def _split_body_waits(self, postordered_blocks):
    """Before lowering the scheduled instruction lists into basic blocks,
    hoist excess sem-waits (>_MAX_WAITS) of any instruction onto same-engine
    nops inserted immediately before it. Same-engine order is preserved, so
    semantics are identical."""
    nc = self.nc
    for insts in postordered_blocks.values():
        needs_fix = any(
            getattr(ins, "sync_info", None) is not None
            and len(ins.sync_info.on_wait) > _MAX_WAITS
            and getattr(ins, "engine", None) is not None
            for ins in insts
        )
        if not needs_fix:
            continue
        out = []
        for ins in insts:
            si = getattr(ins, "sync_info", None)
            if (si is not None and len(si.on_wait) > _MAX_WAITS
                    and getattr(ins, "engine", None) is not None):
                waits = list(si.on_wait)
                excess, keep = waits[:-_MAX_WAITS], waits[-_MAX_WAITS:]
                si.on_wait[:] = keep
                for i in range(0, len(excess), _MAX_WAITS):
                    out.append(mybir.InstNoOp(
                        name=nc.get_next_instruction_name(),
                        engine=ins.engine,
                        bass_nofuse=True,
                        sync_info=mybir.SyncInfo(
                            on_wait=list(excess[i:i + _MAX_WAITS]), on_update=[]),
                    ))
            out.append(ins)
        insts[:] = out
    return tile_mod.TileContext._orig_lower_ordered_insts(self, postordered_blocks)


def _install_drain_fix():
    if getattr(tile_mod.TileContext, "_drain_fix_installed", False):
        return
    tile_mod.TileContext._drain_and_barrier = _split_drain_and_barrier
    tile_mod.TileContext._orig_lower_ordered_insts = (
        tile_mod.TileContext._lower_ordered_insts)
    tile_mod.TileContext._lower_ordered_insts = _split_body_waits
    tile_mod.TileContext._drain_fix_installed = True


def _install_profiling_shims():
    """Best-effort NTFF profiling under axon: the container's `antenv` lacks
    `axon_hooks`, so build it and register the ctypes hook from trn_agent_boot.
    Also neuter the S3 artifact upload (no credentials here)."""
    import sys
    import types

    import concourse.bass_utils as bu
    bu.upload_artifacts = lambda tmpdir: tmpdir

    try:
        import antenv.axon_hooks  # noqa: F401
        return True
    except ImportError:
        pass
    try:
        from trn_agent_boot.trn_boot import _ntff_profile_via_ctypes
        hook = _ntff_profile_via_ctypes("/opt/axon/libaxon_pjrt.so")
    except Exception:
        return False
    if hook is None:
        return False
    m = types.ModuleType("antenv.axon_hooks")
    _state = {"h": hook}
    m.get_axon_ntff_profile_hook = lambda: _state["h"]
    m.set_axon_ntff_profile_hook = lambda h: _state.__setitem__("h", h)
    sys.modules["antenv.axon_hooks"] = m
    import antenv
    antenv.axon_hooks = m
    return True


def _token_tiles(c):
    """Split c tokens into near-equal tiles of <=NT (keeps every matmul's
    moving free dim large enough that LDWEIGHTS stays hidden)."""
    n = -(-c // NT)
    base, rem = divmod(c, n)
    tiles, t0 = [], 0
    for i in range(n):
        tn = base + (1 if i < rem else 0)
        tiles.append((t0, tn))
        t0 += tn
    return tiles


def _block_weights(w):
    """[K, M] -> [M//P, P, K//P, P] where [mo, p, ko, m] = w[ko*P+p, mo*P+m].
    Each [mo] block is one contiguous DMA and one dependency unit, so matmul
    chains can start as soon as their own column block lands."""
    K, M = w.shape
    return np.ascontiguousarray(
        np.asarray(w, np.float32).astype(_BF16)
        .reshape(K // P, P, M // P, P).transpose(2, 1, 0, 3))


_PROG_CACHE = {}


def _get_program(sizes):
    if sizes not in _PROG_CACHE:
        _PROG_CACHE[sizes] = _build_program(sizes)
    return _PROG_CACHE[sizes]


def _build_program(sizes):
    """One SPMD program: shared-expert FFN over TS tokens (full ff), then one
    FFN slot per routed expert over sizes[i] gathered tokens at ff-shard
    width FF_SH. All activations live in [feature, token] layout."""
    _install_drain_fix()
    nc = bass.Bass("TRN2")
    f32, bf16 = mybir.dt.float32, mybir.dt.bfloat16
    SILU = mybir.ActivationFunctionType.Silu

    xs = nc.dram_tensor("xs", [D_MODEL, TS], bf16, kind="ExternalInput")
    ws1 = nc.dram_tensor("ws1", [FO_S, P, KO1, P], bf16, kind="ExternalInput")
    ws3 = nc.dram_tensor("ws3", [FO_S, P, KO1, P], bf16, kind="ExternalInput")
    ws2 = nc.dram_tensor("ws2", [KO1, P, FO_S, P], bf16, kind="ExternalInput")
    ys = nc.dram_tensor("ys", [D_MODEL, TS], bf16, kind="ExternalOutput")
    xr_d, w1_d, w3_d, w2_d, yr_d = [], [], [], [], []
    for i, S in enumerate(sizes):
        xr_d.append(nc.dram_tensor(f"xr{i}", [D_MODEL, S], bf16,
                                   kind="ExternalInput"))
        w1_d.append(nc.dram_tensor(f"w1r{i}", [FO_R, P, KO1, P], bf16,
                                   kind="ExternalInput"))
        w3_d.append(nc.dram_tensor(f"w3r{i}", [FO_R, P, KO1, P], bf16,
                                   kind="ExternalInput"))
        w2_d.append(nc.dram_tensor(f"w2r{i}", [KO1, P, FO_R, P], bf16,
                                   kind="ExternalInput"))
        yr_d.append(nc.dram_tensor(f"yr{i}", [D_MODEL, S], bf16,
                                   kind="ExternalOutput"))

    with tile_mod.TileContext(nc) as tc:
        with (
            tc.tile_pool(name="ws13", bufs=10) as ws13pool,
            tc.tile_pool(name="ws2p", bufs=8) as ws2pool,
            tc.tile_pool(name="wr13", bufs=4) as wr13pool,
            tc.tile_pool(name="wr2p", bufs=16) as wr2pool,
            tc.tile_pool(name="xpool", bufs=3) as xpool,
            tc.tile_pool(name="hspool", bufs=1) as hspool,
            tc.tile_pool(name="hrpool", bufs=2) as hrpool,
            tc.tile_pool(name="spool", bufs=3) as spool,
            tc.tile_pool(name="ypool", bufs=6) as ypool,
            tc.tile_pool(name="p13", bufs=2, space="PSUM") as pspool,
            tc.tile_pool(name="pyp", bufs=4, space="PSUM") as pypool,
        ):
            def ffn(xd, w1d, w3d, w2d, outd, Ct, FO, wpool, w2pool,
                    hpool, htag, w1tag, w3tag, w2tag, first=False):
                xr = xd[:, :].rearrange("(ko p) t -> p ko t", p=P)
                outr = outd[:, :].rearrange("(ko p) t -> p ko t", p=P)
                tiles = _token_tiles(Ct)
                t0_0, tn_0 = tiles[0]
                xt0 = xpool.tile([P, KO1, NT], bf16, tag="xt")
                w1b, w3b, w2b = [], [], []
                if first:
                    # startup-critical chain: first f's weights, then the
                    # first x tile k-sliced so matmul k can start as soon as
                    # slice k lands instead of after the whole 1MB tile
                    t1 = wpool.tile([P, KO1, P], bf16, tag=w1tag)
                    nc.sync.dma_start(t1[:], w1d[0])
                    w1b.append(t1)
                    t3 = wpool.tile([P, KO1, P], bf16, tag=w3tag)
                    nc.sync.dma_start(t3[:], w3d[0])
                    w3b.append(t3)
                    for k in range(KO1):
                        nc.sync.dma_start(xt0[:, k, :tn_0],
                                          xr[:, k, t0_0:t0_0 + tn_0])
                    frange = range(1, FO)
                else:
                    nc.sync.dma_start(xt0[:, :, :tn_0],
                                      xr[:, :, t0_0:t0_0 + tn_0])
                    frange = range(FO)
                for f in frange:
                    t1 = wpool.tile([P, KO1, P], bf16, tag=w1tag)
                    nc.sync.dma_start(t1[:], w1d[f])
                    w1b.append(t1)
                    t3 = wpool.tile([P, KO1, P], bf16, tag=w3tag)
                    nc.sync.dma_start(t3[:], w3d[f])
                    w3b.append(t3)
                if not first:
                    # small (64KB) blocks; needed ~5us into the slot
                    for dch in range(KO1):
                        t2 = w2pool.tile([P, FO, P], bf16, tag=w2tag)
                        nc.sync.dma_start(t2[:], w2d[dch])
                        w2b.append(t2)
                for ti, (t0, tn) in enumerate(tiles):
                    if ti == 0:
                        xt = xt0
                    else:
                        xt = xpool.tile([P, KO1, NT], bf16, tag="xt")
                        nc.sync.dma_start(xt[:, :, :tn], xr[:, :, t0:t0 + tn])
                    h = hpool.tile([P, FO, NT], bf16, tag=htag)
                    for f in range(FO):
                        p1 = pspool.tile([P, NT], f32, tag="p1")
                        p3 = pspool.tile([P, NT], f32, tag="p3")
                        for k in range(KO1):
                            nc.tensor.matmul(
                                p1[:, :tn], w1b[f][:, k, :],
                                xt[:, k, :tn], start=(k == 0), stop=(k == KO1 - 1))
                        for k in range(KO1):
                            nc.tensor.matmul(
                                p3[:, :tn], w3b[f][:, k, :],
                                xt[:, k, :tn], start=(k == 0), stop=(k == KO1 - 1))
                        sl = spool.tile([P, NT], f32, tag="sl")
                        nc.scalar.activation(sl[:, :tn], p1[:, :tn], SILU)
                        nc.vector.tensor_mul(h[:, f, :tn], sl[:, :tn], p3[:, :tn])
                    if first and ti == 0:
                        # stage-2 weights first needed now; keeping their DMAs
                        # behind the stage-1-critical loads preserves startup
                        for dch in range(KO1):
                            t2 = w2pool.tile([P, FO, P], bf16, tag=w2tag)
                            nc.sync.dma_start(t2[:], w2d[dch])
                            w2b.append(t2)
                    for dch in range(KO1):
                        py = pypool.tile([P, NT], f32, tag="py")
                        for f in range(FO):
                            nc.tensor.matmul(
                                py[:, :tn], w2b[dch][:, f, :],
                                h[:, f, :tn], start=(f == 0), stop=(f == FO - 1))
                        yo = ypool.tile([P, NT], bf16, tag="yo")
                        # drain PSUM alternately on vector/scalar so neither
                        # becomes the bottleneck at the 2-matmul py cadence
                        if dch % 2 == 0:
                            nc.vector.tensor_scalar_mul(yo[:, :tn], py[:, :tn], 1.0)
                        else:
                            nc.scalar.copy(yo[:, :tn], py[:, :tn])
                        nc.sync.dma_start(outr[:, dch, t0:t0 + tn], yo[:, :tn])

            ffn(xs, ws1, ws3, ws2, ys, TS, FO_S, ws13pool, ws2pool,
                hspool, "hs", "w1s", "w3s", "w2s", first=True)
            for i, S in enumerate(sizes):
                ffn(xr_d[i], w1_d[i], w3_d[i], w2_d[i], yr_d[i], S, FO_R,
                    wr13pool, wr2pool, hrpool, "hr", "w1r", "w3r", "w2r")
    return nc


def kernel(x, Wg, Ws1, Ws3, Ws2, We1, We3, We2):
    global LAST_EXEC_NS
    x = np.asarray(x)
    xf = np.ascontiguousarray(x.reshape(-1, D_MODEL).astype(np.float32))
    T = xf.shape[0]
    assert T == T_TOKENS, f"kernel compiled for T={T_TOKENS}, got {T}"

    # ---- host routing (gate in fp64; matches the fp32 reference ranking) ----
    logits = xf.astype(np.float64) @ np.asarray(Wg, np.float64)
    gates = 1.0 / (1.0 + np.exp(-logits))
    order = np.argsort(-gates, axis=1, kind="stable")
    idx = order[:, :TOP_K]                                   # [T, 2]
    vals = np.take_along_axis(gates, idx, axis=1)
    w = vals / vals.sum(axis=1, keepdims=True)               # [T, 2]

    tok_lists = [np.where((idx == e).any(axis=1))[0] for e in range(N_ROUTED)]
    loads = np.array([len(t) for t in tok_lists])
    # slots sorted by descending load; smallest slot last = smallest tail
    slot_expert = np.argsort(-loads, kind="stable")
    sizes = tuple(max(64, ((int(loads[e]) + 15) // 16) * 16)
                  for e in slot_expert)

    xf16 = xf.astype(_BF16)
    ws1_b = _block_weights(Ws1)
    ws3_b = _block_weights(Ws3)
    ws2_b = _block_weights(Ws2)

    # per-slot gathered tokens (identical across cores)
    xr_arrs = []
    for i, S in enumerate(sizes):
        tok = tok_lists[slot_expert[i]]
        xg = np.zeros((D_MODEL, S), _BF16)
        xg[:, :len(tok)] = xf16[tok].T
        xr_arrs.append(xg)

    We1 = np.asarray(We1, np.float32)
    We3 = np.asarray(We3, np.float32)
    We2 = np.asarray(We2, np.float32)
    in_maps = []
    for c in range(N_CORES):
        lo, hi = c * FF_SH, (c + 1) * FF_SH
        m = {
            "xs": np.ascontiguousarray(xf16[c * TS:(c + 1) * TS].T),
            "ws1": ws1_b, "ws3": ws3_b, "ws2": ws2_b,
        }
        for i, S in enumerate(sizes):
            e = slot_expert[i]
            m[f"xr{i}"] = xr_arrs[i]
            m[f"w1r{i}"] = _block_weights(We1[e][:, lo:hi])
            m[f"w3r{i}"] = _block_weights(We3[e][:, lo:hi])
            m[f"w2r{i}"] = _block_weights(We2[e][lo:hi, :])
        in_maps.append(m)

    nc = _get_program(sizes)
    profile = bool(int(os.environ.get("KERNEL_PROFILE", "0")))
    if profile:
        profile = _install_profiling_shims()
    try:
        res = run_bass_kernel_spmd(
            nc, in_maps, core_ids=list(range(N_CORES)), trace=profile,
            tmpdir=os.environ.get("KERNEL_TRACE_DIR") or None)
    except Exception:
        # transient device hiccups (e.g. NRT_EXEC_UNIT_UNRECOVERABLE) recover
        # on the next dispatch; retry once without profiling
        res = run_bass_kernel_spmd(
            nc, in_maps, core_ids=list(range(N_CORES)), trace=False)
    LAST_EXEC_NS = res.exec_time_ns
    globals()["LAST_RESULTS"] = res

    out = np.zeros((T, D_MODEL), np.float32)
    for c in range(N_CORES):
        out[c * TS:(c + 1) * TS] = res.results[c]["ys"].T.astype(np.float32)
    for i in range(N_ROUTED):
        e = slot_expert[i]
        tok = tok_lists[e]
        L = len(tok)
        ysum = res.results[0][f"yr{i}"].astype(np.float32)
        for c in range(1, N_CORES):
            ysum += res.results[c][f"yr{i}"].astype(np.float32)
        sel = np.where(idx[tok, 0] == e, w[tok, 0], w[tok, 1]).astype(np.float32)
        out[tok] += ysum[:, :L].T * sel[:, None]
    out *= 1.0 / 3.0
    return out.reshape(x.shape)


# revision 6
# speedup vs baseline: 1.1797x; 1.1797x over previous
"""DeepSeek-style MoE block on 8 Trainium2 NeuronCores.

Sharding strategy (ff-dimension tensor-parallel over the routed experts):
  - Every core holds a 256-wide ff-shard (FF_DIM/8) of ALL 8 routed experts'
    weights and processes EVERY routed (token, expert) pair at 1/8 of the
    FFN work.  Per-core routed work is exactly sum(loads)/8 = 1024 token
    units regardless of the gate's load balance (the old expert-parallel
    layout paid max(load) ~= 1104 per core in padding).
  - The shared expert stays data-parallel: core c processes tokens
    [c*512,(c+1)*512) with the full shared weights.
  - Routed token batches are gathered per expert on host into slots sized
    to the exact per-expert load (rounded to 16), sorted descending so the
    final slot (smallest) minimizes the tail drain.
  - Device emits raw bf16 FFN partials (no gate scaling); the host sums the
    8 ff-shard partials, applies the normalized gate weights and the final
    /(N_SHARED+TOP_K) scale, and scatter-adds.  bf16 partials add ~1e-3
    relative error - well inside the 2e-2 gate (bf16 matmul floor is 4e-3).
  - Matmuls run in bf16 with fp32 PSUM accumulation.
"""

import os

import numpy as np
import ml_dtypes

import concourse.bass as bass
import concourse.mybir as mybir
import concourse.tile as tile_mod
from concourse.bass_utils import run_bass_kernel_spmd
from concourse.vector_clock import ScopedClock

_BF16 = ml_dtypes.bfloat16
P = 128
D_MODEL = 1024
FF_DIM = 2048
N_ROUTED = 8
TOP_K = 2
N_CORES = 8
T_TOKENS = 4096           # 2 * 2048
TS = T_TOKENS // N_CORES  # shared-expert tokens per core
KO1 = D_MODEL // P        # 8  k-chunks for d_model contraction / d_model out
FO_S = FF_DIM // P        # 16 ff chunks for the (full-ff) shared expert
FF_SH = FF_DIM // N_CORES  # 256-wide ff shard per core for routed experts
FO_R = FF_SH // P         # 2  ff chunks per routed slot
NT = 512                  # token tile (matmul moving free dim; one PSUM bank)

LAST_EXEC_NS = None       # set after each kernel() call when profiling


def _split_drain_and_barrier(self, tick_clock, wait_clock):
    """Tile's kernel-tail drain carries one sem-wait per touched engine/queue;
    this walrus build rejects >2 sync waits per instruction. Split the waits
    into single-wait sync nops ahead of the drain (semantically identical:
    the SP stream blocks on each wait in sequence before drain+barrier)."""
    nc = self.nc
    probe = nc.sync.nop(nofuse=True)
    wait_clock.add_sem_waits(probe.ins, ScopedClock({None: tick_clock.global_clock}))
    waits = list(probe.ins.sync_info.on_wait)
    if len(waits) > 1:
        probe.ins.sync_info.on_wait[:] = waits[:1]
        SyncInfo = type(probe.ins.sync_info)
        for w in waits[1:]:
            n2 = nc.sync.nop(nofuse=True)
            n2.ins.sync_info = SyncInfo(on_update=[], on_wait=[w])
    nc.sync.drain()
    nc.all_engine_barrier()
    popped = nc._tile_sem_poison_stack.pop()
    assert popped is self._sem_poison
    nc.clear_and_free_semaphores(list(self.sems.allocated().values()))
    nc.all_engine_barrier()


_MAX_WAITS = 1  # this walrus build rejects multiple sync waits on one instruction


def _split_body_waits(self, postordered_blocks):
    """Before lowering the scheduled instruction lists into basic blocks,
    hoist excess sem-waits (>_MAX_WAITS) of any instruction onto same-engine
    nops inserted immediately before it. Same-engine order is preserved, so
    semantics are identical."""
    nc = self.nc
    for insts in postordered_blocks.values():
        needs_fix = any(
            getattr(ins, "sync_info", None) is not None
            and len(ins.sync_info.on_wait) > _MAX_WAITS
            and getattr(ins, "engine", None) is not None
            for ins in insts
        )
        if not needs_fix:
            continue
        out = []
        for ins in insts:
            si = getattr(ins, "sync_info", None)
            if (si is not None and len(si.on_wait) > _MAX_WAITS
                    and getattr(ins, "engine", None) is not None):
                waits = list(si.on_wait)
                excess, keep = waits[:-_MAX_WAITS], waits[-_MAX_WAITS:]
                si.on_wait[:] = keep
                for i in range(0, len(excess), _MAX_WAITS):
                    out.append(mybir.InstNoOp(
                        name=nc.get_next_instruction_name(),
                        engine=ins.engine,
                        bass_nofuse=True,
                        sync_info=mybir.SyncInfo(
                            on_wait=list(excess[i:i + _MAX_WAITS]), on_update=[]),
                    ))
            out.append(ins)
        insts[:] = out
    return tile_mod.TileContext._orig_lower_ordered_insts(self, postordered_blocks)


def _install_drain_fix():
    if getattr(tile_mod.TileContext, "_drain_fix_installed", False):
        return
    tile_mod.TileContext._drain_and_barrier = _split_drain_and_barrier
    tile_mod.TileContext._orig_lower_ordered_insts = (
        tile_mod.TileContext._lower_ordered_insts)
    tile_mod.TileContext._lower_ordered_insts = _split_body_waits
    tile_mod.TileContext._drain_fix_installed = True


def _install_profiling_shims():
    """Best-effort NTFF profiling under axon: the container's `antenv` lacks
    `axon_hooks`, so build it and register the ctypes hook from trn_agent_boot.
    Also neuter the S3 artifact upload (no credentials here)."""
    import sys
    import types

    import concourse.bass_utils as bu
    bu.upload_artifacts = lambda tmpdir: tmpdir

    try:
        import antenv.axon_hooks  # noqa: F401
        return True
    except ImportError:
        pass
    try:
        from trn_agent_boot.trn_boot import _ntff_profile_via_ctypes
        hook = _ntff_profile_via_ctypes("/opt/axon/libaxon_pjrt.so")
    except Exception:
        return False
    if hook is None:
        return False
    m = types.ModuleType("antenv.axon_hooks")
    _state = {"h": hook}
    m.get_axon_ntff_profile_hook = lambda: _state["h"]
    m.set_axon_ntff_profile_hook = lambda h: _state.__setitem__("h", h)
    sys.modules["antenv.axon_hooks"] = m
    import antenv
    antenv.axon_hooks = m
    return True


def _token_tiles(c):
    """Split c tokens into near-equal tiles of <=NT (keeps every matmul's
    moving free dim large enough that LDWEIGHTS stays hidden)."""
    n = -(-c // NT)
    base, rem = divmod(c, n)
    tiles, t0 = [], 0
    for i in range(n):
        tn = base + (1 if i < rem else 0)
        tiles.append((t0, tn))
        t0 += tn
    return tiles


def _block_weights(w):
    """[K, M] -> [M//P, P, K//P, P] where [mo, p, ko, m] = w[ko*P+p, mo*P+m].
    Each [mo] block is one contiguous DMA and one dependency unit, so matmul
    chains can start as soon as their own column block lands."""
    K, M = w.shape
    return np.ascontiguousarray(
        np.asarray(w, np.float32).astype(_BF16)
        .reshape(K // P, P, M // P, P).transpose(2, 1, 0, 3))


_PROG_CACHE = {}


def _get_program(sizes):
    if sizes not in _PROG_CACHE:
        _PROG_CACHE[sizes] = _build_program(sizes)
    return _PROG_CACHE[sizes]


def _build_program(sizes):
    """One SPMD program: shared-expert FFN over TS tokens (full ff), then one
    FFN slot per routed expert over sizes[i] gathered tokens at ff-shard
    width FF_SH. All activations live in [feature, token] layout."""
    _install_drain_fix()
    nc = bass.Bass("TRN2")
    f32, bf16 = mybir.dt.float32, mybir.dt.bfloat16
    SILU = mybir.ActivationFunctionType.Silu

    xs = nc.dram_tensor("xs", [D_MODEL, TS], bf16, kind="ExternalInput")
    ws1 = nc.dram_tensor("ws1", [FO_S, P, KO1, P], bf16, kind="ExternalInput")
    ws3 = nc.dram_tensor("ws3", [FO_S, P, KO1, P], bf16, kind="ExternalInput")
    ws2 = nc.dram_tensor("ws2", [KO1, P, FO_S, P], bf16, kind="ExternalInput")
    ys = nc.dram_tensor("ys", [D_MODEL, TS], bf16, kind="ExternalOutput")
    xr_d, w1_d, w3_d, w2_d, yr_d = [], [], [], [], []
    for i, S in enumerate(sizes):
        xr_d.append(nc.dram_tensor(f"xr{i}", [D_MODEL, S], bf16,
                                   kind="ExternalInput"))
        w1_d.append(nc.dram_tensor(f"w1r{i}", [FO_R, P, KO1, P], bf16,
                                   kind="ExternalInput"))
        w3_d.append(nc.dram_tensor(f"w3r{i}", [FO_R, P, KO1, P], bf16,
                                   kind="ExternalInput"))
        w2_d.append(nc.dram_tensor(f"w2r{i}", [KO1, P, FO_R, P], bf16,
                                   kind="ExternalInput"))
        yr_d.append(nc.dram_tensor(f"yr{i}", [D_MODEL, S], bf16,
                                   kind="ExternalOutput"))

    with tile_mod.TileContext(nc) as tc:
        with (
            tc.tile_pool(name="ws13", bufs=10) as ws13pool,
            tc.tile_pool(name="ws2p", bufs=8) as ws2pool,
            tc.tile_pool(name="wr13", bufs=2) as wr13pool,
            tc.tile_pool(name="wr2p", bufs=2) as wr2pool,
            tc.tile_pool(name="xpool", bufs=3) as xpool,
            tc.tile_pool(name="hspool", bufs=1) as hspool,
            tc.tile_pool(name="hrpool", bufs=2) as hrpool,
            tc.tile_pool(name="spool", bufs=3) as spool,
            tc.tile_pool(name="ypool", bufs=3) as ypool,
            tc.tile_pool(name="p13", bufs=2, space="PSUM") as pspool,
            tc.tile_pool(name="pyp", bufs=4, space="PSUM") as pypool,
        ):
            # ---- slot descriptors: shared expert first, routed slots after
            slots = []
            slots.append(dict(
                xr=xs[:, :].rearrange("(ko p) t -> p ko t", p=P),
                outr=ys[:, :].rearrange("(ko p) t -> p ko t", p=P),
                w1d=ws1, w3d=ws3, w2d=ws2, FO=FO_S,
                wpool=ws13pool, w2pool=ws2pool, hpool=hspool,
                htag="hs", w1tag="w1s", w3tag="w3s", w2tag="w2s",
                tiles=_token_tiles(TS), first=True))
            for i, S in enumerate(sizes):
                slots.append(dict(
                    xr=xr_d[i][:, :].rearrange("(ko p) t -> p ko t", p=P),
                    outr=yr_d[i][:, :].rearrange("(ko p) t -> p ko t", p=P),
                    w1d=w1_d[i], w3d=w3_d[i], w2d=w2_d[i], FO=FO_R,
                    wpool=wr13pool, w2pool=wr2pool, hpool=hrpool,
                    htag="hr", w1tag="w1r", w3tag="w3r", w2tag="w2r",
                    tiles=_token_tiles(S), first=False))

            def slot_inputs(s):
                """Emit the slot's x tile 0 + all weight DMAs (sync engine).
                These never wait on compute beyond ring-buffer backpressure."""
                t0_0, tn_0 = s["tiles"][0]
                xt0 = xpool.tile([P, KO1, NT], bf16, tag="xt")
                s["xts"] = {0: xt0}
                if s["first"]:
                    # startup-critical: first f's weights, then x tile 0
                    # k-sliced so matmul k starts as soon as slice k lands
                    w1b, w3b = [], []
                    t1 = s["wpool"].tile([P, KO1, P], bf16, tag=s["w1tag"])
                    nc.sync.dma_start(t1[:], s["w1d"][0])
                    w1b.append(t1)
                    t3 = s["wpool"].tile([P, KO1, P], bf16, tag=s["w3tag"])
                    nc.sync.dma_start(t3[:], s["w3d"][0])
                    w3b.append(t3)
                    for k in range(KO1):
                        nc.sync.dma_start(xt0[:, k, :tn_0],
                                          s["xr"][:, k, t0_0:t0_0 + tn_0])
                    for f in range(1, s["FO"]):
                        t1 = s["wpool"].tile([P, KO1, P], bf16, tag=s["w1tag"])
                        nc.sync.dma_start(t1[:], s["w1d"][f])
                        w1b.append(t1)
                        t3 = s["wpool"].tile([P, KO1, P], bf16, tag=s["w3tag"])
                        nc.sync.dma_start(t3[:], s["w3d"][f])
                        w3b.append(t3)
                    s["w1"] = lambda f, k: w1b[f][:, k, :]
                    s["w3"] = lambda f, k: w3b[f][:, k, :]
                    w2b = []
                    for dch in range(KO1):
                        t2 = s["w2pool"].tile([P, s["FO"], P], bf16,
                                              tag=s["w2tag"])
                        nc.sync.dma_start(t2[:], s["w2d"][dch])
                        w2b.append(t2)
                    s["w2"] = lambda dch, f: w2b[dch][:, f, :]
                else:
                    nc.sync.dma_start(xt0[:, :, :tn_0],
                                      s["xr"][:, :, t0_0:t0_0 + tn_0])
                    FO = s["FO"]
                    w1t = s["wpool"].tile([P, FO, KO1, P], bf16, tag=s["w1tag"])
                    nc.sync.dma_start(
                        w1t[:], s["w1d"][:].rearrange("f p k m -> p f k m"))
                    w3t = s["wpool"].tile([P, FO, KO1, P], bf16, tag=s["w3tag"])
                    nc.sync.dma_start(
                        w3t[:], s["w3d"][:].rearrange("f p k m -> p f k m"))
                    w2t = s["w2pool"].tile([P, KO1, FO, P], bf16, tag=s["w2tag"])
                    nc.sync.dma_start(
                        w2t[:], s["w2d"][:].rearrange("d p f m -> p d f m"))
                    s["w1"] = lambda f, k: w1t[:, f, k, :]
                    s["w3"] = lambda f, k: w3t[:, f, k, :]
                    s["w2"] = lambda dch, f: w2t[:, dch, f, :]

            def xt_input(s, ti):
                t0, tn = s["tiles"][ti]
                xt = xpool.tile([P, KO1, NT], bf16, tag="xt")
                s["xts"][ti] = xt
                nc.sync.dma_start(xt[:, :, :tn], s["xr"][:, :, t0:t0 + tn])

            def tile_compute(s, ti):
                t0, tn = s["tiles"][ti]
                xt = s["xts"].pop(ti)
                FO = s["FO"]
                h = s["hpool"].tile([P, FO, NT], bf16, tag=s["htag"])
                for f in range(FO):
                    p1 = pspool.tile([P, NT], f32, tag="p1")
                    p3 = pspool.tile([P, NT], f32, tag="p3")
                    for k in range(KO1):
                        nc.tensor.matmul(
                            p1[:, :tn], s["w1"](f, k),
                            xt[:, k, :tn], start=(k == 0), stop=(k == KO1 - 1))
                    for k in range(KO1):
                        nc.tensor.matmul(
                            p3[:, :tn], s["w3"](f, k),
                            xt[:, k, :tn], start=(k == 0), stop=(k == KO1 - 1))
                    sl = spool.tile([P, NT], f32, tag="sl")
                    nc.scalar.activation(sl[:, :tn], p1[:, :tn], SILU)
                    nc.vector.tensor_mul(h[:, f, :tn], sl[:, :tn], p3[:, :tn])
                yo = ypool.tile([P, KO1, NT], bf16, tag="yo")
                for dch in range(KO1):
                    py = pypool.tile([P, NT], f32, tag="py")
                    for f in range(FO):
                        nc.tensor.matmul(
                            py[:, :tn], s["w2"](dch, f),
                            h[:, f, :tn], start=(f == 0), stop=(f == FO - 1))
                    # drain PSUM alternately on vector/scalar so neither
                    # becomes the bottleneck at the 2-matmul py cadence
                    if dch % 2 == 0:
                        nc.vector.tensor_scalar_mul(
                            yo[:, dch, :tn], py[:, :tn], 1.0)
                    else:
                        nc.scalar.copy(yo[:, dch, :tn], py[:, :tn])
                outr = s["outr"]
                return lambda: nc.sync.dma_start(
                    outr[:, :, t0:t0 + tn], yo[:, :, :tn])

            # Flat schedule.  Each tile's output DMA is emitted one tile
            # LATE in the sync stream, and the next slot's input DMAs are
            # hoisted ahead of it (at the current slot's last tile), so the
            # output DMA's wait-on-cast never head-of-line-blocks prefetch.
            slot_inputs(slots[0])
            pending = None
            for si, s in enumerate(slots):
                ntile = len(s["tiles"])
                for ti in range(ntile):
                    if ti + 1 < ntile:
                        xt_input(s, ti + 1)
                    if ti == ntile - 1 and si + 1 < len(slots):
                        slot_inputs(slots[si + 1])
                    if pending is not None:
                        pending()
                    pending = tile_compute(s, ti)
            pending()
    return nc


def kernel(x, Wg, Ws1, Ws3, Ws2, We1, We3, We2):
    global LAST_EXEC_NS
    x = np.asarray(x)
    xf = np.ascontiguousarray(x.reshape(-1, D_MODEL).astype(np.float32))
    T = xf.shape[0]
    assert T == T_TOKENS, f"kernel compiled for T={T_TOKENS}, got {T}"

    # ---- host routing (gate in fp64; matches the fp32 reference ranking) ----
    logits = xf.astype(np.float64) @ np.asarray(Wg, np.float64)
    gates = 1.0 / (1.0 + np.exp(-logits))
    order = np.argsort(-gates, axis=1, kind="stable")
    idx = order[:, :TOP_K]                                   # [T, 2]
    vals = np.take_along_axis(gates, idx, axis=1)
    w = vals / vals.sum(axis=1, keepdims=True)               # [T, 2]

    tok_lists = [np.where((idx == e).any(axis=1))[0] for e in range(N_ROUTED)]
    loads = np.array([len(t) for t in tok_lists])
    # slots sorted by descending load; smallest slot last = smallest tail
    slot_expert = np.argsort(-loads, kind="stable")
    sizes = tuple(max(64, ((int(loads[e]) + 15) // 16) * 16)
                  for e in slot_expert)

    xf16 = xf.astype(_BF16)
    ws1_b = _block_weights(Ws1)
    ws3_b = _block_weights(Ws3)
    ws2_b = _block_weights(Ws2)

    # per-slot gathered tokens (identical across cores)
    xr_arrs = []
    for i, S in enumerate(sizes):
        tok = tok_lists[slot_expert[i]]
        xg = np.zeros((D_MODEL, S), _BF16)
        xg[:, :len(tok)] = xf16[tok].T
        xr_arrs.append(xg)

    We1 = np.asarray(We1, np.float32)
    We3 = np.asarray(We3, np.float32)
    We2 = np.asarray(We2, np.float32)
    in_maps = []
    for c in range(N_CORES):
        lo, hi = c * FF_SH, (c + 1) * FF_SH
        m = {
            "xs": np.ascontiguousarray(xf16[c * TS:(c + 1) * TS].T),
            "ws1": ws1_b, "ws3": ws3_b, "ws2": ws2_b,
        }
        for i, S in enumerate(sizes):
            e = slot_expert[i]
            m[f"xr{i}"] = xr_arrs[i]
            m[f"w1r{i}"] = _block_weights(We1[e][:, lo:hi])
            m[f"w3r{i}"] = _block_weights(We3[e][:, lo:hi])
            m[f"w2r{i}"] = _block_weights(We2[e][lo:hi, :])
        in_maps.append(m)

    nc = _get_program(sizes)
    profile = bool(int(os.environ.get("KERNEL_PROFILE", "0")))
    if profile:
        profile = _install_profiling_shims()
    try:
        res = run_bass_kernel_spmd(
            nc, in_maps, core_ids=list(range(N_CORES)), trace=profile,
            tmpdir=os.environ.get("KERNEL_TRACE_DIR") or None)
    except Exception:
        # transient device hiccups (e.g. NRT_EXEC_UNIT_UNRECOVERABLE) recover
        # on the next dispatch; retry once without profiling
        res = run_bass_kernel_spmd(
            nc, in_maps, core_ids=list(range(N_CORES)), trace=False)
    LAST_EXEC_NS = res.exec_time_ns
    globals()["LAST_RESULTS"] = res

    out = np.zeros((T, D_MODEL), np.float32)
    for c in range(N_CORES):
        out[c * TS:(c + 1) * TS] = res.results[c]["ys"].T.astype(np.float32)
    for i in range(N_ROUTED):
        e = slot_expert[i]
        tok = tok_lists[e]
        L = len(tok)
        ysum = res.results[0][f"yr{i}"].astype(np.float32)
        for c in range(1, N_CORES):
            ysum += res.results[c][f"yr{i}"].astype(np.float32)
        sel = np.where(idx[tok, 0] == e, w[tok, 0], w[tok, 1]).astype(np.float32)
        out[tok] += ysum[:, :L].T * sel[:, None]
    out *= 1.0 / 3.0
    return out.reshape(x.shape)
